# revision 12
# baseline (speedup 1.0000x reference)
"""Trainium2 Bass kernel for nn_Cache retrieval-KNN.

Problem (per reference):
    query (64, 1, 8, 1024) f32, keys (512, 8, 65536) f32
    q = query -> (ql=64, b=8, h=1024); K = keys -> (b, n=512, j=64, h=1024)
    scores[i,b,n] = max_j  q[i,b,:] . K[b,n,j,:]
    attention = softmax(scores / 32, axis=n)              -> (64, 8, 512) f32
    topk_indices = top_k(attention, 4) permuted (k, ql, b) -> (4, 64, 8) i32

Strategy:
  * Data-parallel over batch: core b gets its own keys[:, b, :] slice.
  * Keys/queries are converted to fp16 on the host (products are exact in
    fp32 PSUM accumulation; score abs-noise ~1e-4 -> attention relative
    error ~3e-5, far inside tolerance), which halves HBM streaming
    (64 MiB/core) -- the kernel is HBM-bound at ~165us of streaming.
  * Device: PE matmuls (fp16, q stationary / keys moving), segmented
    max-pool on DVE straight out of PSUM, then softmax (ACT exp with
    sum-accumulate) and Max8/MaxIndex for the top-8 candidates.  The
    last key slab is split into 512-row chunks so the tail drains fast.
  * fp16 noise could reorder near-tied top-4 candidates, so the host
    re-ranks each row's device top-8 with an exact fp64 recompute (0.5
    GFLOP total) -- final indices match the fp32/fp64 reference exactly.
"""

import numpy as np

import concourse.bass as bass
import concourse.mybir as mybir
from concourse.tile import TileContext
from concourse.bass_utils import run_bass_kernel_spmd

QL, BSZ, CACHE_L, CACHE_N, NHID = 64, 8, 64, 512, 1024
TOPK = 4
N_CORES = 8
ROWS = CACHE_N * CACHE_L          # 32768 flattened (n, j) key rows per batch
N_RB = 16                         # row-block (slab) count
COLS = ROWS // N_RB               # 2048 key rows per slab
HB = NHID // 128                  # 8 contraction chunks of 128
NSUB = COLS // 512                # 4 PSUM-bank matmuls per slab
SCALE = float(1.0 / np.float32(np.sqrt(np.float32(NHID))))  # 1/32

MM_DT = mybir.dt.float16
NP_MM = np.float16


def _split_multi_waits(nc, max_waits=1):
    """walrus in this container only accepts one sync-wait per instruction;
    split extra waits onto preceding same-engine no-ops."""
    for f in nc.m.functions:
        for blk in f.blocks:
            out = []
            changed = False
            for inst in blk.instructions:
                si = inst.sync_info
                waits = list(si.on_wait) if (si is not None and si.on_wait) else []
                if len(waits) > max_waits:
                    head, tail = waits[:-max_waits], waits[-max_waits:]
                    for w in head:
                        nop = mybir.InstNoOp(
                            name=f"I-wsplit-{nc.next_id()}", ins=[], outs=[]
                        )
                        nop.engine = inst.engine
                        nop.sync_info = mybir.SyncInfo(on_wait=[w], on_update=[])
                        out.append(nop)
                    si.on_wait = tail
                    changed = True
                out.append(inst)
            if changed:
                blk.instructions = out


def _build():
    nc = bass.Bass()
    kt = nc.declare_dram_parameter(
        "kt", [N_RB, 128, HB * COLS], MM_DT, isOutput=False
    )
    qt = nc.declare_dram_parameter("qt", [128, HB * QL], MM_DT, isOutput=False)
    att_o = nc.declare_dram_parameter(
        "att", [QL, CACHE_N], mybir.dt.float32, isOutput=True
    )
    idx_o = nc.declare_dram_parameter("idx", [QL, 8], mybir.dt.uint32, isOutput=True)

    with TileContext(nc) as tc:
        with (
            tc.tile_pool(name="singles", bufs=1) as singles,
            tc.tile_pool(name="slabs", bufs=4) as slabs,
            tc.tile_pool(name="tails", bufs=4) as tails,
            tc.tile_pool(name="psp", bufs=8, space="PSUM") as psp,
            tc.tile_pool(name="ep", bufs=1) as ep,
        ):
            qsb = singles.tile([128, HB, QL], MM_DT)
            nc.scalar.dma_start(out=qsb, in_=qt.rearrange("p (k m) -> p k m", k=HB))
            pooled = singles.tile([QL, CACHE_N], mybir.dt.float32)

            def score_block(rhs_sl, lo):
                ps = psp.tile([QL, 512], mybir.dt.float32, tag="ps")
                for k in range(HB):
                    nc.tensor.matmul(
                        ps,
                        lhsT=qsb[:, k, :],
                        rhs=rhs_sl(k),
                        start=(k == 0),
                        stop=(k == HB - 1),
                    )
                nc.vector.reduce_max(
                    out=pooled[:, lo : lo + 8],
                    in_=ps.rearrange("p (g j) -> p g j", j=CACHE_L),
                    axis=mybir.AxisListType.X,
                )

            kt_v = [kt[rb].rearrange("p (k c) -> p k c", k=HB) for rb in range(N_RB)]
            for rb in range(N_RB - 1):
                ksb = slabs.tile([128, HB, COLS], MM_DT, tag="ksb")
                nc.sync.dma_start(out=ksb, in_=kt_v[rb])
                for sub in range(NSUB):
                    score_block(
                        lambda k, ksb=ksb, sub=sub: ksb[
                            :, k, sub * 512 : (sub + 1) * 512
                        ],
                        rb * 32 + sub * 8,
                    )
            # last slab in 512-row chunks so the tail drains quickly
            base = (N_RB - 1) * 32
            for sub in range(NSUB):
                kc = tails.tile([128, HB, 512], MM_DT, tag="kchunk")
                nc.sync.dma_start(
                    out=kc, in_=kt_v[N_RB - 1][:, :, sub * 512 : (sub + 1) * 512]
                )
                score_block(lambda k, kc=kc: kc[:, k, :], base + sub * 8)

            # unnormalized: att = exp(pooled/32).  pooled/32 is in
            # [-0.1, 0.1], so no max-subtraction is needed for stability;
            # the host divides by the row sum.  Runs on ACT concurrently
            # with the DVE top-8 below.
            att = ep.tile([QL, CACHE_N], mybir.dt.float32)
            nc.scalar.activation(
                att, pooled, mybir.ActivationFunctionType.Exp, scale=SCALE
            )
            nc.sync.dma_start(out=att_o[:], in_=att)

            # top-8 (values descending) + their indices
            m8 = ep.tile([QL, 8], mybir.dt.float32)
            nc.vector.max(out=m8, in_=pooled)
            idx8 = ep.tile([QL, 8], mybir.dt.uint32)
            nc.vector.max_index(out=idx8, in_max=m8, in_values=pooled)
            nc.scalar.dma_start(out=idx_o[:], in_=idx8)

    return nc


_NC_CACHE = {}


def _get_nc():
    if "nc" not in _NC_CACHE:
        _NC_CACHE["nc"] = _build()
    return _NC_CACHE["nc"]


def _prep_inputs(query, keys16):
    """Per-core input dicts. kt layout: [rb, p, hb*COLS+c] =
    K_flat[rb*COLS + c, hb*128 + p]; qt: [p, hb*QL+i] = q[i, hb*128+p]."""
    in_maps = []
    for b in range(BSZ):
        qb = query[:, 0, b, :]  # (64, 1024) f32
        qt = (
            qb.T.reshape(HB, 128, QL)
            .transpose(1, 0, 2)
            .reshape(128, HB * QL)
        )
        qt = np.ascontiguousarray(qt, dtype=NP_MM)
        kf = keys16[:, b, :].reshape(N_RB, COLS, HB, 128)
        kt = np.ascontiguousarray(kf.transpose(0, 3, 2, 1)).reshape(
            N_RB, 128, HB * COLS
        )
        in_maps.append({"kt": kt, "qt": qt})
    return in_maps


def _exact_rerank(query, keys, cand_per_core):
    """Re-rank each row's candidate slots with exact fp64 scores."""
    out = np.empty((TOPK, QL, BSZ), np.int32)
    for b in range(BSZ):
        qb = query[:, 0, b, :].astype(np.float64)          # (64, 1024)
        kb = keys[:, b, :]                                  # (512, 65536) f32
        cand = cand_per_core[b].astype(np.int64)            # (64, ncand)
        nc_ = cand.shape[1]
        kc = kb[cand.reshape(-1)].reshape(QL, nc_ * CACHE_L, NHID).astype(np.float64)
        s = np.matmul(kc, qb[:, :, None])[..., 0]           # (64, nc*64)
        s = s.reshape(QL, nc_, CACHE_L).max(axis=2)         # exact pooled
        for i in range(QL):
            c, sc = cand[i], s[i]
            c, keep = np.unique(c, return_index=True)
            sc = sc[keep]
            order = np.lexsort((c, -sc))                    # desc, ties: low idx
            out[:, i, b] = c[order[:TOPK]].astype(np.int32)
    return out


def _run(in_maps, **kw):
    nc = _get_nc()
    if not _NC_CACHE.get("split"):
        # CoreSim can't execute the injected no-ops, so split only for HW.
        _split_multi_waits(nc)
        _NC_CACHE["split"] = True
    return run_bass_kernel_spmd(nc, in_maps, list(range(N_CORES)), **kw)


def kernel(query, keys, _res_out=None, **run_kw):
    query = np.ascontiguousarray(np.asarray(query), dtype=np.float32)
    keys = np.ascontiguousarray(np.asarray(keys), dtype=np.float32)
    keys16 = keys.astype(NP_MM)
    in_maps = _prep_inputs(query, keys16)
    res = _run(in_maps, **run_kw)
    if _res_out is not None:
        _res_out.append(res)
    att_exp = np.stack(
        [res.results[b]["att"] for b in range(BSZ)], axis=1
    )                                                       # (64, 8, 512)
    att = (att_exp / att_exp.sum(axis=2, keepdims=True)).astype(np.float32)
    cand_per_core = [res.results[b]["idx"] for b in range(BSZ)]
    topk = _exact_rerank(query, keys, cand_per_core)        # (4, 64, 8) int32
    return att, topk


# revision 17
# speedup vs baseline: 1.1400x; 1.1400x over previous
"""Trainium2 Bass kernel for nn_Cache retrieval-KNN.

Problem (per reference):
    query (64, 1, 8, 1024) f32, keys (512, 8, 65536) f32
    q = query -> (ql=64, b=8, h=1024); K = keys -> (b, n=512, j=64, h=1024)
    scores[i,b,n] = max_j  q[i,b,:] . K[b,n,j,:]
    attention = softmax(scores / 32, axis=n)              -> (64, 8, 512) f32
    topk_indices = top_k(attention, 4) permuted (k, ql, b) -> (4, 64, 8) i32

Strategy:
  * Data-parallel over batch: core b gets its own keys[:, b, :] slice.
  * Keys/queries are converted to fp16 on the host (products are exact in
    fp32 PSUM accumulation; score abs-noise ~1e-4 -> attention relative
    error ~3e-5, far inside tolerance), which halves HBM streaming
    (64 MiB/core) -- the kernel is HBM-bound at ~165us of streaming.
  * Device: PE matmuls (fp16, q stationary / keys moving), segmented
    max-pool on DVE straight out of PSUM, then softmax (ACT exp with
    sum-accumulate) and Max8/MaxIndex for the top-8 candidates.  The
    last key slab is split into 512-row chunks so the tail drains fast.
  * fp16 noise could reorder near-tied top-4 candidates, so the host
    re-ranks each row's device top-8 with an exact fp64 recompute (0.5
    GFLOP total) -- final indices match the fp32/fp64 reference exactly.
"""

import numpy as np

import concourse.bass as bass
import concourse.mybir as mybir
from concourse.tile import TileContext
from concourse.bass_utils import run_bass_kernel_spmd

QL, BSZ, CACHE_L, CACHE_N, NHID = 64, 8, 64, 512, 1024
TOPK = 4
N_CORES = 8
ROWS = CACHE_N * CACHE_L          # 32768 flattened (n, j) key rows per batch
N_RB = 16                         # row-block (slab) count
COLS = ROWS // N_RB               # 2048 key rows per slab
HB = NHID // 128                  # 8 contraction chunks of 128
NSUB = COLS // 512                # 4 PSUM-bank matmuls per slab
NCAND = 12                        # host re-rank candidate count
SCALE = float(1.0 / np.float32(np.sqrt(np.float32(NHID))))  # 1/32

MM_DT = mybir.dt.float16
NP_MM = np.float16


def _split_multi_waits(nc, max_waits=1):
    """walrus in this container only accepts one sync-wait per instruction;
    split extra waits onto preceding same-engine no-ops."""
    for f in nc.m.functions:
        for blk in f.blocks:
            out = []
            changed = False
            for inst in blk.instructions:
                si = inst.sync_info
                waits = list(si.on_wait) if (si is not None and si.on_wait) else []
                if len(waits) > max_waits:
                    head, tail = waits[:-max_waits], waits[-max_waits:]
                    for w in head:
                        nop = mybir.InstNoOp(
                            name=f"I-wsplit-{nc.next_id()}", ins=[], outs=[]
                        )
                        nop.engine = inst.engine
                        nop.sync_info = mybir.SyncInfo(on_wait=[w], on_update=[])
                        out.append(nop)
                    si.on_wait = tail
                    changed = True
                out.append(inst)
            if changed:
                blk.instructions = out


def _build():
    nc = bass.Bass()
    kt = nc.declare_dram_parameter(
        "kt", [N_RB, 128, HB * COLS], MM_DT, isOutput=False
    )
    qt = nc.declare_dram_parameter("qt", [128, HB * QL], MM_DT, isOutput=False)
    att_o = nc.declare_dram_parameter(
        "att", [QL, CACHE_N], mybir.dt.float32, isOutput=True
    )

    with TileContext(nc) as tc:
        with (
            tc.tile_pool(name="singles", bufs=1) as singles,
            tc.tile_pool(name="slabs", bufs=4) as slabs,
            tc.tile_pool(name="tails", bufs=4) as tails,
            tc.tile_pool(name="psp", bufs=8, space="PSUM") as psp,
            tc.tile_pool(name="ep", bufs=1) as ep,
        ):
            qsb = singles.tile([128, HB, QL], MM_DT)
            nc.scalar.dma_start(out=qsb, in_=qt.rearrange("p (k m) -> p k m", k=HB))
            pooled = singles.tile([QL, CACHE_N], mybir.dt.float32)

            def score_block(rhs_sl, lo):
                ps = psp.tile([QL, 512], mybir.dt.float32, tag="ps")
                for k in range(HB):
                    nc.tensor.matmul(
                        ps,
                        lhsT=qsb[:, k, :],
                        rhs=rhs_sl(k),
                        start=(k == 0),
                        stop=(k == HB - 1),
                    )
                nc.vector.reduce_max(
                    out=pooled[:, lo : lo + 8],
                    in_=ps.rearrange("p (g j) -> p g j", j=CACHE_L),
                    axis=mybir.AxisListType.X,
                )

            kt_v = [kt[rb].rearrange("p (k c) -> p k c", k=HB) for rb in range(N_RB)]
            for rb in range(N_RB - 1):
                ksb = slabs.tile([128, HB, COLS], MM_DT, tag="ksb")
                nc.sync.dma_start(out=ksb, in_=kt_v[rb])
                for sub in range(NSUB):
                    score_block(
                        lambda k, ksb=ksb, sub=sub: ksb[
                            :, k, sub * 512 : (sub + 1) * 512
                        ],
                        rb * 32 + sub * 8,
                    )
            # last slab in 512-row chunks so the tail drains quickly
            base = (N_RB - 1) * 32
            for sub in range(NSUB):
                kc = tails.tile([128, HB, 512], MM_DT, tag="kchunk")
                nc.sync.dma_start(
                    out=kc, in_=kt_v[N_RB - 1][:, :, sub * 512 : (sub + 1) * 512]
                )
                score_block(lambda k, kc=kc: kc[:, k, :], base + sub * 8)

            # unnormalized: att = exp(pooled/32).  pooled/32 is in
            # [-0.1, 0.1], so no max-subtraction is needed for stability;
            # the host divides by the row sum (and picks top-k candidates
            # from att, since exp is monotone).
            att = ep.tile([QL, CACHE_N], mybir.dt.float32)
            nc.scalar.activation(
                att, pooled, mybir.ActivationFunctionType.Exp, scale=SCALE
            )
            nc.sync.dma_start(out=att_o[:], in_=att)

    return nc


_NC_CACHE = {}


def _get_nc():
    if "nc" not in _NC_CACHE:
        _NC_CACHE["nc"] = _build()
    return _NC_CACHE["nc"]


def _prep_inputs(query, keys16):
    """Per-core input dicts. kt layout: [rb, p, hb*COLS+c] =
    K_flat[rb*COLS + c, hb*128 + p]; qt: [p, hb*QL+i] = q[i, hb*128+p]."""
    in_maps = []
    for b in range(BSZ):
        qb = query[:, 0, b, :]  # (64, 1024) f32
        qt = (
            qb.T.reshape(HB, 128, QL)
            .transpose(1, 0, 2)
            .reshape(128, HB * QL)
        )
        qt = np.ascontiguousarray(qt, dtype=NP_MM)
        kf = keys16[:, b, :].reshape(N_RB, COLS, HB, 128)
        kt = np.ascontiguousarray(kf.transpose(0, 3, 2, 1)).reshape(
            N_RB, 128, HB * COLS
        )
        in_maps.append({"kt": kt, "qt": qt})
    return in_maps


def _exact_rerank(query, keys, cand_per_core):
    """Re-rank each row's candidate slots with exact fp64 scores."""
    out = np.empty((TOPK, QL, BSZ), np.int32)
    for b in range(BSZ):
        qb = query[:, 0, b, :].astype(np.float64)          # (64, 1024)
        kb = keys[:, b, :]                                  # (512, 65536) f32
        cand = np.clip(cand_per_core[b].astype(np.int64), 0, CACHE_N - 1)
        nc_ = cand.shape[1]
        kc = kb[cand.reshape(-1)].reshape(QL, nc_ * CACHE_L, NHID).astype(np.float64)
        s = np.matmul(kc, qb[:, :, None])[..., 0]           # (64, nc*64)
        s = s.reshape(QL, nc_, CACHE_L).max(axis=2)         # exact pooled
        for i in range(QL):
            c, sc = cand[i], s[i]
            c, keep = np.unique(c, return_index=True)
            sc = sc[keep]
            order = np.lexsort((c, -sc))                    # desc, ties: low idx
            out[:, i, b] = c[order[:TOPK]].astype(np.int32)
    return out


def _run(in_maps, **kw):
    nc = _get_nc()
    if not _NC_CACHE.get("split"):
        # CoreSim can't execute the injected no-ops, so split only for HW.
        _split_multi_waits(nc)
        _NC_CACHE["split"] = True
    return run_bass_kernel_spmd(nc, in_maps, list(range(N_CORES)), **kw)


def kernel(query, keys, _res_out=None, **run_kw):
    query = np.ascontiguousarray(np.asarray(query), dtype=np.float32)
    keys = np.ascontiguousarray(np.asarray(keys), dtype=np.float32)
    keys16 = keys.astype(NP_MM)
    in_maps = _prep_inputs(query, keys16)
    res = _run(in_maps, **run_kw)
    if _res_out is not None:
        _res_out.append(res)
    att_exp = np.stack(
        [res.results[b]["att"] for b in range(BSZ)], axis=1
    )                                                       # (64, 8, 512)
    att = (att_exp / att_exp.sum(axis=2, keepdims=True)).astype(np.float32)
    # top candidates per (i, b) from the device scores (exp is monotone)
    cand = np.argpartition(-att_exp, NCAND - 1, axis=2)[:, :, :NCAND]
    topk = _exact_rerank(query, keys, [cand[:, b, :] for b in range(BSZ)])
    return att, topk
